# revision 29
# baseline (speedup 1.0000x reference)
"""CBOW model (embedding gather -> mean -> logits -> softmax) on 8 Trainium2
NeuronCores.

Sharding (vocab/model parallel for the matmul + softmax, per the hint; the
gather is batch-sharded against a replicated W1 to keep the random-row DMA
traffic tiny and off the critical path):
  - W2 is sharded along vocab: core m owns columns [m*12500, (m+1)*12500)
    (fp16, padded to 12544).  W1 is replicated (fp16) and each core gathers
    only ITS two batch tiles (256 rows x 10 ctx) -> 0.65 MB of random reads
    per core instead of 5.2 MB.  A 2-chunk AllGather (even tiles / odd
    tiles) gives every core the full transposed hidden [D, 2048] ~50us in.
  - Softmax in a SINGLE matmul/exp pass per batch tile: matmul chunks
    produce f32 logits in PSUM; ACT computes exp -> SBUF (bf16, kept
    resident in one of 6 rotating buffers) with fused per-row partial sums
    (accum_out); DVE/ACT copy the f32 logits to f16 per-group staging tiles
    that DMA straight out.  Per 2-tile chunk the row sums are AllReduced
    (the collective stream is otherwise idle after the AllGathers, and a
    dummy warmup collective absorbs the first-collective latency); 1/Z
    comes from DVE reciprocal and the resident exp values are scaled in
    place (DVE 4x tensor_scalar, per-partition scalar) and DMA'd out as
    softmax (bf16) on the ACT HWDGE ring, two tiles behind the compute
    wavefront.  No second matmul or exp pass.  Max-subtraction is not
    needed: |logit| < 40 always, exp fits bf16 comfortably.
"""

import numpy as np

import concourse.bass as bass
import concourse.mybir as mybir
import concourse.tile as tile
from concourse import bacc
from concourse.masks import make_identity
import concourse.bass_utils as bass_utils

# Problem shape (hardcoded; matches the CBOW reference).
V = 100000      # vocab
D = 128         # embed dim
B = 2048        # batch
C = 10          # context positions
M = 8           # cores
S = V // M      # vocab shard per core = 12500
SP = 12544      # shard padded so every matmul chunk is >= 256 wide
P = 128         # partitions
BT = B // P     # batch tiles = 16
TPC = BT // M   # batch tiles gathered per core = 2
MMN = 512       # max moving free dim per matmul (one PSUM bank, f32)
GRP = 2048      # vocab columns per PSUM group (4 banks)

F32 = mybir.dt.float32
F16 = mybir.dt.float16
BF16 = mybir.dt.bfloat16
I32 = mybir.dt.int32
AF = mybir.ActivationFunctionType

# (start, width) vocab-column groups per core; width <= GRP.  The pad columns
# beyond S are never computed past the matmul (exp/copy/DMA use gwS).
GROUPS = [(g0, min(GRP, SP - g0)) for g0 in range(0, SP, GRP)]
# groups whose PSUM->SBUF logits cast runs on the scalar engine instead of
# DVE, to balance the two engines (ACT also does every group's exp).
ACT_COPY_GROUPS = {len(GROUPS) - 1, len(GROUPS) - 2}
SCH = 2          # batch tiles per softmax-sum AllReduce
NSC = BT // SCH
EBUFS = 6        # resident exp buffers (SBUF budget-limited)


def build_nc(n_cores: int = M):
    nc = bacc.Bacc("TRN2", target_bir_lowering=False, debug=False,
                   num_devices=n_cores)

    w1f = nc.dram_tensor("w1f", [V, D], F16, kind="ExternalInput")
    w2s = nc.dram_tensor("w2s", [P, SP], F16, kind="ExternalInput")
    idxs = nc.dram_tensor("idxs", [P, TPC * C], I32, kind="ExternalInput")
    logits_s = nc.dram_tensor("logits_s", [B, S], F16, kind="ExternalOutput")
    soft_s = nc.dram_tensor("soft_s", [B, S], BF16, kind="ExternalOutput")

    rg = [list(range(n_cores))]
    shared = "Shared" if n_cores > 1 else "Local"

    with tile.TileContext(nc) as tc:
        with tc.tile_pool(name="sbuf", bufs=1) as sbuf, \
             tc.tile_pool(name="gathp", bufs=2) as gathp, \
             tc.tile_pool(name="hidp", bufs=2) as hidp, \
             tc.tile_pool(name="stagp", bufs=4) as stagp, \
             tc.tile_pool(name="ebufp", bufs=EBUFS) as ebufp, \
             tc.tile_pool(name="psum", bufs=2, space="PSUM") as psum, \
             tc.tile_pool(name="dram", bufs=1, space="DRAM") as dram:
            idx_sb = sbuf.tile([P, TPC * C], I32)
            nc.sync.dma_start(out=idx_sb[:], in_=idxs[:])

            ident = sbuf.tile([P, P], F16)
            make_identity(nc, ident[:])

            # W2 shard resident in SBUF for the whole kernel.
            w2_sb = sbuf.tile([P, SP], F16)
            nc.sync.dma_start(out=w2_sb[:], in_=w2s[:])

            if n_cores > 1:
                # Warm up the collectives stream with a dummy tiny AllReduce
                # so the first real collective doesn't pay the ~45us
                # first-collective latency.
                warm_sb = sbuf.tile([P, 2], F32)
                nc.gpsimd.memset(warm_sb[:], 0.0)
                warm_in = dram.tile([P, 2], F32, name="warm_in")
                warm_out = dram.tile([P, 2], F32, name="warm_out",
                                     addr_space="Shared")
                nc.gpsimd.dma_start(out=warm_in[:], in_=warm_sb[:])
                nc.gpsimd.collective_compute(
                    "AllReduce", mybir.AluOpType.add, replica_groups=rg,
                    ins=[warm_in[:]], outs=[warm_out[:]],
                )

            # ---- gather my TPC batch tiles, build transposed hidden ----
            hch = sbuf.tile([P, TPC * P], F16)   # [D, tt*128+p]
            for tt in range(TPC):
                gath = gathp.tile([P, C * D], F16, tag="gath")
                for c in range(C):
                    j = tt * C + c
                    nc.gpsimd.indirect_dma_start(
                        out=gath[:, c * D:(c + 1) * D],
                        out_offset=None,
                        in_=w1f[:],
                        in_offset=bass.IndirectOffsetOnAxis(
                            ap=idx_sb[:, j:j + 1], axis=0),
                    )
                hidf = hidp.tile([P, D], F32, tag="hidf")
                nc.vector.tensor_reduce(
                    out=hidf[:],
                    in_=gath[:].rearrange("p (c d) -> p d c", c=C),
                    axis=mybir.AxisListType.X,
                    op=mybir.AluOpType.add,
                )
                hid16 = hidp.tile([P, D], F16, tag="hid16")
                # context mean folded in here (x 1/10)
                nc.vector.tensor_scalar_mul(hid16[:], hidf[:], 1.0 / C)
                tp = psum.tile([P, 2 * GRP], F16, tag="mm")
                nc.tensor.transpose(out=tp[:, :P], in_=hid16[:],
                                    identity=ident[:])
                nc.vector.tensor_copy(hch[:, tt * P:(tt + 1) * P],
                                      tp[:, :P])

            # ---- AllGather the hidden in 2 chunks (even tiles, odd tiles).
            # Core m's chunk lands in block m: cc_out[(m*P+d), p] =
            # hidden[d, (TPC*m + tt)*128 + p].
            assert n_cores in (1, M)
            hidT = []
            for tt in range(TPC):
                cc_in = dram.tile([P, P], F16, name=f"hag_in{tt}")
                cc_out = dram.tile([M * P, P], F16, name=f"hag_out{tt}",
                                   addr_space=shared)
                nc.gpsimd.dma_start(out=cc_in[:],
                                    in_=hch[:, tt * P:(tt + 1) * P])
                if n_cores > 1:
                    nc.gpsimd.collective_compute(
                        "AllGather", mybir.AluOpType.bypass,
                        replica_groups=rg,
                        ins=[cc_in[:]], outs=[cc_out[:]],
                    )
                else:
                    # debug build: only block 0 is real; tiles 2..15 garbage
                    nc.gpsimd.dma_start(out=cc_out[:P, :], in_=cc_in[:])
                ht = sbuf.tile([P, M * P], F16, name=f"hidT{tt}")
                hidT.append((ht, cc_out))

            def read_hidT(tt):
                ht, cc_out = hidT[tt]
                nc.sync.dma_start(
                    out=ht[:].rearrange("d (m p) -> d m p", m=M),
                    in_=cc_out[:].rearrange("(m d) p -> d m p", m=M))

            def lhsT_of(t):
                # tile t = TPC*m + tt -> chunk tt, block m
                return hidT[t % TPC][0][:, (t // TPC) * P:(t // TPC + 1) * P]

            ebufs = [None] * BT
            lsum = sbuf.tile([P, BT], F32)
            gsums = [None] * NSC
            # pending per-group softmax-scale pieces, interleaved one per
            # matmul group so DVE's in-order queue never sees a 3.5us burst
            # that would delay the PSUM-releasing casts (which stall the PE).
            pend_scale = []

            def pass1_tile(t):
                lhsT = lhsT_of(t)
                eb = ebufp.tile([P, SP], BF16, tag="eb")
                ebufs[t] = eb
                sums = hidp.tile([P, len(GROUPS)], F32, tag="sums")
                for gi, (g0, gw) in enumerate(GROUPS):
                    gwS = min(gw, S - g0)   # drop the zero-pad columns
                    ps = psum.tile([P, GRP], F32, tag="mm")
                    for s0 in range(0, gw, MMN):
                        w = min(MMN, gw - s0)
                        nc.tensor.matmul(
                            out=ps[:, s0:s0 + w], lhsT=lhsT,
                            rhs=w2_sb[:, g0 + s0:g0 + s0 + w],
                            start=True, stop=True)
                    nc.scalar.activation(
                        out=eb[:, g0:g0 + gwS], in_=ps[:, :gwS], func=AF.Exp,
                        accum_out=sums[:, gi:gi + 1])
                    stag = stagp.tile([P, GRP], F16, tag="stag")
                    if gi in ACT_COPY_GROUPS:
                        nc.scalar.copy(stag[:, :gwS], ps[:, :gwS])
                    else:
                        nc.vector.tensor_copy(stag[:, :gwS], ps[:, :gwS])
                    nc.sync.dma_start(
                        out=logits_s[t * P:(t + 1) * P, g0:g0 + gwS],
                        in_=stag[:, :gwS])
                    if pend_scale:
                        pend_scale.pop(0)()
                nc.vector.tensor_reduce(
                    out=lsum[:, t:t + 1], in_=sums[:],
                    axis=mybir.AxisListType.X, op=mybir.AluOpType.add)

            def sum_allreduce(j):
                h0 = j * SCH
                cc_s_in = dram.tile([P, SCH], F32, name=f"ccsi{j}")
                cc_s_out = dram.tile([P, SCH], F32, name=f"ccso{j}",
                                     addr_space=shared)
                nc.gpsimd.dma_start(out=cc_s_in[:],
                                    in_=lsum[:, h0:h0 + SCH])
                if n_cores > 1:
                    nc.gpsimd.collective_compute(
                        "AllReduce", mybir.AluOpType.add, replica_groups=rg,
                        ins=[cc_s_in[:]], outs=[cc_s_out[:]],
                    )
                else:
                    nc.gpsimd.dma_start(out=cc_s_out[:], in_=cc_s_in[:])
                gsum = sbuf.tile([P, SCH], F32, name=f"gsum{j}")
                nc.gpsimd.dma_start(out=gsum[:], in_=cc_s_out[:])
                gsums[j] = gsum

            def queue_scale_chunk(j):
                """Queue chunk j's scaling as per-group pieces (popped one
                per matmul group of later tiles) + the softmax DMAs.  The
                reciprocal rides in the first piece so it lands in DVE's
                queue well after the AllReduce it waits on has completed."""
                zinv = sbuf.tile([P, SCH], F32, name=f"zinv{j}")
                for tt in range(SCH):
                    t = j * SCH + tt
                    eb = ebufs[t]
                    for gi, (g0, gw) in enumerate(GROUPS):
                        gwS = min(gw, S - g0)

                        def piece(eb=eb, g0=g0, gwS=gwS, tt=tt, zinv=zinv,
                                  t=t, j=j,
                                  first=(tt == 0 and gi == 0),
                                  last=(gi == len(GROUPS) - 1)):
                            if first:
                                nc.vector.reciprocal(zinv[:], gsums[j][:])
                            nc.vector.tensor_scalar_mul(
                                eb[:, g0:g0 + gwS], eb[:, g0:g0 + gwS],
                                zinv[:, tt:tt + 1])
                            if last:
                                nc.scalar.dma_start(
                                    out=soft_s[t * P:(t + 1) * P, :],
                                    in_=eb[:, :S])
                        pend_scale.append(piece)

            def flush_scales():
                while pend_scale:
                    pend_scale.pop(0)()

            # ---- emission ----
            read_hidT(0)
            for t in range(BT):
                pass1_tile(t)
                if t == 0 and TPC > 1:
                    read_hidT(1)
                if t % SCH == 1:
                    sum_allreduce(t // SCH)
                elif t >= 2:
                    # chunk j-1's AllReduce was emitted one tile ago; its
                    # pieces start draining during the NEXT tile, by which
                    # point the collective has comfortably finished.
                    queue_scale_chunk(t // SCH - 1)
            queue_scale_chunk(NSC - 1)
            flush_scales()

    nc.compile()
    return nc


def make_in_maps(inputs: np.ndarray, W1: np.ndarray, W2: np.ndarray,
                 n_cores: int = M):
    idx = np.asarray(inputs).astype(np.int64)
    w1m = np.asarray(W1, dtype=np.float32).astype(np.float16)
    W2 = np.asarray(W2, dtype=np.float32)
    in_maps = []
    for m in range(n_cores):
        lo = m * S
        # tiles TPC*m .. TPC*m+TPC-1 of the batch, global vocab indices
        rows = idx[TPC * m * P:TPC * (m + 1) * P]          # [TPC*128, C]
        idxm = np.ascontiguousarray(
            rows.astype(np.int32).reshape(TPC, P, C)
            .transpose(1, 0, 2).reshape(P, TPC * C))
        w2m = np.zeros((P, SP), np.float16)
        w2m[:, :S] = W2[:, lo:lo + S]
        in_maps.append({"w1f": w1m, "w2s": w2m, "idxs": idxm})
    return in_maps


_NC_CACHE = {}


def kernel(inputs: np.ndarray, W1: np.ndarray, W2: np.ndarray):
    if "nc" not in _NC_CACHE:
        _NC_CACHE["nc"] = build_nc(M)
    nc = _NC_CACHE["nc"]
    in_maps = make_in_maps(inputs, W1, W2, M)
    res = bass_utils.run_bass_kernel_spmd(nc, in_maps, core_ids=list(range(M)))
    logits = np.empty((B, V), np.float32)
    soft = np.empty((B, V), np.float32)
    for m in range(M):
        logits[:, m * S:(m + 1) * S] = np.asarray(
            res.results[m]["logits_s"]).astype(np.float32)
        soft[:, m * S:(m + 1) * S] = np.asarray(
            res.results[m]["soft_s"]).astype(np.float32)
    return logits, soft
